# revision 1
# baseline (speedup 1.0000x reference)
"""Self-contained TRN2 Bass kernel for nn_A3Net (2-layer spiking net, LIF neurons).

kernel(x, W1, W2) -> spikes (T=20, B=262144, C=4) float32, bit-exact vs the
jax reference.

Strategy (pure data parallel: batch sharded across 8 NeuronCores, 32768 each):

  Reference recurrence per batch element (T=20 steps, beta=0.9, thresh=1):
    mem1_t = 0.9*mem1_{t-1} + W1 @ x_t - spk1_{t-1};  spk1_t = [mem1_t > 1]
    mem2_t = 0.9*mem2_{t-1} + W2 @ spk1_t - spk2_{t-1};  spk2_t = [mem2_t > 1]

  Rescaling by alpha^t (alpha = 1/0.9) turns the leaky integration into pure
  accumulation, which the TensorEngine performs in fp32 PSUM across all steps:
    u1 = sum_{tau<=t} alpha^tau (W1 x_tau - spk1_{tau-1}),  mem1_t>1 <=> u1 > alpha^t
    w2 = sum_{tau<=t} alpha^tau (W2 spk1_{tau-1} - spk2_{tau-2}),
                                               mem2_{t-1}>1 <=> w2 > alpha^t
  Spikes are carried as s = spk * alpha^t, so ONE fused VectorE tensor_scalar
  (is_gt alpha^t, mult alpha^t) per step computes both layers' spikes straight
  out of PSUM, and the stationary matrix is constant across steps.

  Per (chunk of 2048 batch, step): ONE block-diagonal 128x128 fp32 matmul
  (4 batch groups per column). Partition layout (rhs rows == psum rows, so the
  compare needs no partition shift):
    0..95   s1 spikes (group g at 24g), 96..111 s2 spikes (4g) -> one
    contiguous out-DMA block, 112..123 x rows (host-prescaled by alpha^t).
  8 chunks in flight = 8 PSUM banks (two 4-bank quads, compares span 4 banks
  in one instruction), 2 sequential passes over the 16 chunks.

  Host: prescale x by alpha^t + data layout; convert raw spike values (>0) to
  1.0 afterwards. Measured on HW: ~430 us per invocation (TensorE-bound:
  fp32 matmuls stream at 4 cycles/column on TRN2).
"""
import sys
sys.path.insert(0, '/opt/trn_rl_repo')
import numpy as np
import concourse.mybir as mybir
from concourse import bacc
from concourse.tile import TileContext
from concourse.bass_utils import run_bass_kernel_spmd

T = 20
F, H, C = 3, 24, 4
G = 4
N = 512
NQ = 4 * N
M = G * (H + C)     # 112
S2_0 = G * H        # 96
KX0 = M             # 112
KEND = M + G * F    # 124
N_CORES = 8
B_CORE = 32768
N_CHUNKS = 16
DT = mybir.dt.float32

_cache = {}


def _scales():
    return np.array([(1.0 / 0.9) ** t for t in range(T + 2)],
                    dtype=np.float64).astype(np.float32)


def _build_stationary(W1, W2, last=False):
    a = np.float64(1.0) / np.float64(0.9)
    A = np.zeros((KEND, M), dtype=np.float64)
    for g in range(G):
        for h in range(H):
            A[24 * g + h, 24 * g + h] = -a                      # reset1
            for c in range(C):
                A[24 * g + h, S2_0 + 4 * g + c] = a * W2[c, h]  # layer2
        for c in range(C):
            A[S2_0 + 4 * g + c, S2_0 + 4 * g + c] = -a          # reset2
        if not last:
            for f in range(F):
                for h in range(H):
                    A[KX0 + 3 * g + f, 24 * g + h] = W1[h, f]   # W1 x
    return np.ascontiguousarray(A.astype(np.float32))


def _build_nc(repeat=1, mm_dtype=DT):
    n_pass, chunks_per_pass, quads_per_pass = 2, 8, 2
    nc = bacc.Bacc("TRN2", target_bir_lowering=False, num_devices=N_CORES)
    xs_e = nc.declare_dram_parameter("xs", [T, N_CHUNKS, G * F, N], DT, isOutput=False)
    out_e = nc.declare_dram_parameter("out", [T, N_CHUNKS, G * C, N], DT, isOutput=True)
    a01_e = nc.declare_dram_parameter("A01", [KEND, M], DT, isOutput=False)
    alast_e = nc.declare_dram_parameter("Alast", [KEND, M], DT, isOutput=False)
    th = _scales()

    with TileContext(nc) as tc:
        with (
            tc.tile_pool(name="wpool", bufs=1) as wpool,
            tc.tile_pool(name="spk", bufs=4) as spool,
            tc.tile_pool(name="psum", bufs=1, space="PSUM") as ppool,
        ):
            A01 = wpool.tile([KEND, M], DT)
            Alast = wpool.tile([KEND, M], DT)
            nc.sync.dma_start(A01[:], a01_e[:])
            nc.sync.dma_start(Alast[:], alast_e[:])
            for rep in range(repeat):
              for p in range(n_pass):
                psums = [ppool.tile([128, NQ], DT, name=f"ps_r{rep}p{p}_q{q}",
                                    tag=f"ps{q}", bufs=1)
                         for q in range(quads_per_pass)]
                cur = []
                for q in range(quads_per_pass):
                    c0 = p * chunks_per_pass + q * 4
                    tile = spool.tile([128, NQ], DT, name=f"s_r{rep}p{p}q{q}t0",
                                      tag=f"spk{q}")
                    nc.gpsimd.memset(tile[0:M, :], 0.0)
                    nc.sync.dma_start(
                        tile[KX0:KEND, :].rearrange("p (c n) -> p c n", c=4),
                        xs_e[0, c0:c0 + 4].rearrange("c p n -> p c n"))
                    cur.append(tile)
                for t in range(T + 1):
                    nxt_list = []
                    for q in range(quads_per_pass):
                        c0 = p * chunks_per_pass + q * 4
                        lhs = Alast if t == T else A01
                        for c in range(4):
                            nc.tensor.matmul(
                                psums[q][0:M, N * c:N * (c + 1)],
                                lhs[:, :].bitcast(mm_dtype),
                                cur[q][0:KEND, N * c:N * (c + 1)].bitcast(mm_dtype),
                                start=(t == 0), stop=(t == T),
                            )
                        nxt = spool.tile([128, NQ], DT,
                                         name=f"s_r{rep}p{p}q{q}t{t + 1}",
                                         tag=f"spk{q}")
                        if t + 1 < T:
                            nc.sync.dma_start(
                                nxt[KX0:KEND, :].rearrange("p (c n) -> p c n", c=4),
                                xs_e[t + 1, c0:c0 + 4].rearrange("c p n -> p c n"))
                        thr = float(th[t])
                        nc.vector.tensor_scalar(
                            nxt[0:M, :], psums[q][0:M, :], thr, thr,
                            mybir.AluOpType.is_gt, mybir.AluOpType.mult)
                        if t >= 1:
                            src = nxt[S2_0:S2_0 + G * C, :].rearrange(
                                "p (c n) -> p c n", c=4)
                            dst = out_e[t - 1, c0:c0 + 4].rearrange(
                                "c p n -> p c n")
                            nc.gpsimd.dma_start(dst, src)
                        nxt_list.append(nxt)
                    cur = nxt_list
    nc.compile()
    return nc


def _host_prep(x, W1, W2):
    th = _scales()
    A01 = _build_stationary(W1, W2, last=False)
    Alast = _build_stationary(W1, W2, last=True)
    xs_full = x * th[:T, None, None]          # prescale by alpha^t
    in_maps = []
    for k in range(N_CORES):
        xk = xs_full[:, k * B_CORE:(k + 1) * B_CORE, :]
        xk = xk.reshape(T, N_CHUNKS, G, N, F).transpose(0, 1, 2, 4, 3)
        xk = np.ascontiguousarray(xk.reshape(T, N_CHUNKS, G * F, N))
        in_maps.append({"xs": xk, "A01": A01, "Alast": Alast})
    return in_maps


def _host_post(results):
    outs = []
    for k in range(N_CORES):
        o = results[k]["out"]
        o = o.reshape(T, N_CHUNKS, G, C, N).transpose(0, 1, 2, 4, 3)
        outs.append(o.reshape(T, B_CORE, C))
    full = np.concatenate(outs, axis=1)
    return (full > 0).astype(np.float32)


def kernel(x, W1, W2):
    x = np.asarray(x, dtype=np.float32)
    W1 = np.asarray(W1, dtype=np.float32)
    W2 = np.asarray(W2, dtype=np.float32)
    assert x.shape == (T, N_CORES * B_CORE, F), x.shape

    if "nc" not in _cache:
        _cache["nc"] = _build_nc()
    nc = _cache["nc"]

    in_maps = _host_prep(x, W1, W2)
    res = run_bass_kernel_spmd(nc, in_maps, core_ids=list(range(N_CORES)))
    return _host_post(res.results)



# revision 2
# speedup vs baseline: 66.5841x; 66.5841x over previous
"""Self-contained TRN2 Bass kernel for nn_A3Net (2-layer spiking net, LIF).

kernel(x, W1, W2) -> spikes (T=20, B=262144, C=4) float32.

Strategy (pure data parallel: batch sharded across 8 NeuronCores, 32768
each; per-core IO is ~10.7MB vs the chunked baseline's ~29MB):

  - Recurrence rescaled by alpha^t (alpha = 1/0.9) so the leaky
    integration becomes pure PSUM accumulation; spikes are carried as
    s = spk * alpha^t and ONE fused VectorE tensor_scalar (is_gt alpha^t,
    mult alpha^t) per step computes both layers' spikes out of PSUM.
  - The alpha^t prescale of x is folded into per-step stationary matrices
    A_t (rows 112..123 = alpha^t * W1; zero at t=20), so x ships raw and
    only ~163KB of W-derived data goes to each core.
  - x ships as (T, 4, F, 8192) float32 planes per core (one threaded host
    transpose); each (t, quad) load is a single 12-partition DMA with
    8KB-contiguous runs.  b = quad*8192 + g*2048 + j.
  - Output leaves as (T, 4, C, 8192) uint8 planes (4x fewer D2H bytes
    than f32): layer-2 PSUM rows are cc-major (96 + 4*cc + g) so each
    (t, quad) store is a single 16-partition DMA with 2KB runs; an extra
    uint8 is_gt per (t, quad) produces clean 0/1.  Host merges planes
    into (T, B, 4) float32 with a threaded cast-transpose.
  - 2 passes x 2 quads of 8192 batch: one [128,2048] fp32 PSUM tile per
    quad (4 batch groups per 512-column matmul block); TensorE streams
    one quad while VectorE thresholds the other.

Row layout: layer1 24g+h; layer2 96+4cc+g; x rows 112+4f+g.
"""
import sys
sys.path.insert(0, '/opt/trn_rl_repo')
import numpy as np
import concourse.mybir as mybir
from concourse import bacc
from concourse.tile import TileContext
from concourse.bass_utils import run_bass_kernel_spmd
from concurrent.futures import ThreadPoolExecutor

T = 20
F, H, C = 3, 24, 4
G = 4
QB = 8192           # batch per quad
NQ = QB // G        # 2048 psum columns per quad
N = 512             # columns per matmul (one PSUM bank)
M = G * (H + C)     # 112
S2_0 = G * H        # 96
KX0 = M             # 112
KEND = M + G * F    # 124
N_CORES = 8
N_QUAD = 4
B_CORE = N_QUAD * QB
DT = mybir.dt.float32
OUT_DT = mybir.dt.uint8

_cache = {}
_pool = ThreadPoolExecutor(max_workers=N_CORES)


def _scales():
    return (np.float64(1.0) / np.float64(0.9)) ** np.arange(T + 2)


def _build_A(W1, W2):
    """Ab [112,112] recurrent block; Ax [21,12,112] per-step x-blocks."""
    a = np.float64(1.0) / np.float64(0.9)
    Ab = np.zeros((M, M), np.float64)
    W1_ = np.asarray(W1, np.float64)
    W2_ = np.asarray(W2, np.float64)
    for g in range(G):
        for h in range(H):
            Ab[H * g + h, H * g + h] = -a                       # reset1
            for cc in range(C):
                Ab[H * g + h, S2_0 + C * cc + g] = a * W2_[cc, h]
        for cc in range(C):
            Ab[S2_0 + C * cc + g, S2_0 + C * cc + g] = -a       # reset2
    th = _scales()
    Ax = np.zeros((T + 1, G * F, M), np.float64)
    for t in range(T):
        for f in range(F):
            for g in range(G):
                Ax[t, G * f + g, H * g:H * g + H] = th[t] * W1_[:, f]
    return (np.ascontiguousarray(Ab.astype(np.float32)),
            np.ascontiguousarray(Ax.astype(np.float32)))


def _build_nc(repeat=1, n_pass=2, quads_per_pass=2):
    n_quad = n_pass * quads_per_pass
    b_core = n_quad * QB
    nc = bacc.Bacc("TRN2", target_bir_lowering=False, num_devices=N_CORES)
    xq_e = nc.declare_dram_parameter("xq", [T, n_quad, F, QB], DT,
                                     isOutput=False)
    ab_e = nc.declare_dram_parameter("Ab", [M, M], DT, isOutput=False)
    ax_e = nc.declare_dram_parameter("Ax", [T + 1, G * F, M], DT,
                                     isOutput=False)
    out_e = nc.declare_dram_parameter("out", [T, n_quad, C, QB], OUT_DT,
                                      isOutput=True)
    th = _scales().astype(np.float32)

    with TileContext(nc) as tc:
        with (
            tc.tile_pool(name="wpool", bufs=1) as wpool,
            tc.tile_pool(name="spk", bufs=4) as spool,
            tc.tile_pool(name="o01", bufs=3) as opool,
            tc.tile_pool(name="psum", bufs=1, space="PSUM") as ppool,
        ):
            Aall = wpool.tile([KEND, (T + 1) * M], DT)
            for t in range(T + 1):
                nc.sync.dma_start(Aall[0:M, M * t:M * (t + 1)], ab_e[:])
            nc.sync.dma_start(
                Aall[KX0:KEND, :].rearrange("p (t m) -> p t m", t=T + 1),
                ax_e[:].rearrange("t p m -> p t m"))
            for rep in range(repeat):
              for p in range(n_pass):
                psums = [ppool.tile([128, NQ], DT, name=f"ps_r{rep}p{p}_q{q}",
                                    tag=f"ps{q}", bufs=1)
                         for q in range(quads_per_pass)]
                cur = []
                for q in range(quads_per_pass):
                    qq = p * quads_per_pass + q
                    tile = spool.tile([128, NQ], DT, name=f"s_r{rep}p{p}q{q}t0",
                                      tag=f"spk{q}")
                    nc.gpsimd.memset(tile[0:M, :], 0.0)
                    nc.sync.dma_start(
                        tile[KX0:KEND, :],
                        xq_e[0, qq].rearrange("f (g j) -> (f g) j", g=G))
                    cur.append(tile)
                for t in range(T + 1):
                    lhs = Aall[:, M * t:M * (t + 1)]
                    thr = float(th[t])
                    nxt_list = []
                    for q in range(quads_per_pass):
                        qq = p * quads_per_pass + q
                        for c in range(4):
                            nc.tensor.matmul(
                                psums[q][0:M, N * c:N * (c + 1)],
                                lhs,
                                cur[q][0:KEND, N * c:N * (c + 1)],
                                start=(t == 0), stop=(t == T),
                                skip_group_check=True,
                            )
                        if t < T:
                            nxt = spool.tile([128, NQ], DT,
                                             name=f"s_r{rep}p{p}q{q}t{t + 1}",
                                             tag=f"spk{q}")
                            if t + 1 < T:
                                nc.sync.dma_start(
                                    nxt[KX0:KEND, :],
                                    xq_e[t + 1, qq].rearrange(
                                        "f (g j) -> (f g) j", g=G))
                            else:
                                # last step consumes no x (A_20 x-block is
                                # 0); zero rows 96:128 (32-aligned start;
                                # the tensor_scalar re-writes 96:112)
                                nc.vector.memset(nxt[S2_0:128, :], 0.0)
                            nc.vector.tensor_scalar(
                                nxt[0:M, :], psums[q][0:M, :], thr, thr,
                                mybir.AluOpType.is_gt, mybir.AluOpType.mult)
                            nxt_list.append(nxt)
                        if t >= 1:
                            o01 = opool.tile([16, NQ], OUT_DT,
                                             name=f"o_r{rep}p{p}q{q}t{t}",
                                             tag=f"o01{q}")
                            nc.vector.tensor_scalar(
                                o01[:], psums[q][S2_0:S2_0 + 16, :],
                                thr, None, mybir.AluOpType.is_gt)
                            # psum row 96+4cc+g, col j -> b = qq*QB+g*2048+j
                            nc.scalar.dma_start(
                                out_e[t - 1, qq].rearrange(
                                    "c (g j) -> (c g) j", g=G),
                                o01[:])
                    cur = nxt_list
    nc.compile()
    return nc


def _host_prep(x, W1, W2, b_core=B_CORE, n_cores=N_CORES):
    Ab, Ax = _build_A(W1, W2)
    n_quad = b_core // QB

    def mk(k):
        xk = x[:, k * b_core:(k + 1) * b_core, :]
        xq = np.ascontiguousarray(
            xk.reshape(T, n_quad, QB, F).transpose(0, 1, 3, 2))
        return {"xq": xq, "Ab": Ab, "Ax": Ax}

    return list(_pool.map(mk, range(n_cores)))


def _host_post(results, b_core=B_CORE, n_cores=N_CORES):
    n_quad = b_core // QB
    full = np.empty((T, n_cores * b_core, C), np.float32)

    def merge(k):
        o = results[k]["out"]          # (T, n_quad, C, QB) uint8
        for qq in range(n_quad):
            b0 = k * b_core + qq * QB
            full[:, b0:b0 + QB, :] = o[:, qq].transpose(0, 2, 1)

    list(_pool.map(merge, range(n_cores)))
    return full


def kernel(x, W1, W2):
    x = np.asarray(x, dtype=np.float32)
    W1 = np.asarray(W1, dtype=np.float32)
    W2 = np.asarray(W2, dtype=np.float32)
    assert x.shape == (T, N_CORES * B_CORE, F), x.shape

    if "nc" not in _cache:
        _cache["nc"] = _build_nc()
    nc = _cache["nc"]

    in_maps = _host_prep(x, W1, W2)
    res = run_bass_kernel_spmd(nc, in_maps, core_ids=list(range(N_CORES)))
    return _host_post(res.results)
